# revision 29
# baseline (speedup 1.0000x reference)
"""Adaptive Huber/MSE/L1 loss on 8 TRN2 NeuronCores (Bass/Tile).

Reference math (per sample, N = 4,096,000 elements):
    e   = pred - true
    L2  = mean(e^2);  L1 = mean(|e|)
    huber_elem = where(|e| <= 5, 0.5 e^2, 5(|e| - 2.5))
               = 0.5 e^2 - 0.5 relu(|e| - 5)^2
    huber = (S2 - SR) * 0.5 / N        (S2 = sum e^2, SR = sum relu(|e|-5)^2)
    use_l2 = (L2 <= 1) | (L2 < L1^2)
    loss = mean_over_batch(where(use_l2, L2, huber))

Sharding: data-parallel, sample i -> core i. Each core reduces its
32.8 MB shard to three sums and applies the branch locally; the host
averages the 8 per-sample scalars during unshard (an on-device 4-byte
AllReduce costs ~42 us of pure latency).

Per-tile engine split, sized so every engine sits ~20% under the
~358 GB/s-per-core DMA floor (5.86 us per 2 MB tile pair):
    DVE : e = a - b;  |e| = e & 0x7fffffff (uint32 bitcast, 2x mode);
          m = max(|e|,5) - 5 (fused tensor_scalar, 2x mode)
    ACT : Square(e) + row-accum (S2);  Square(m) + row-accum (SR)
    PE  : ones^T @ |e| chunks accumulated in PSUM (S1), plus the final
          [P,2] -> [1,2] partition reduction
The trailing columns use small tiles so the last dependency chain is
short and the kernel tail hugs the final DMA.

Hardware pitfalls baked in: DVE tensor_tensor_reduce and tensor_scalar
abs_max/accum_out fail on this toolchain (avoided); GpSimd elementwise
runs ~30 us/tile AND port-starves DVE (avoided); profiling must capture
all 8 devices (see test harness).
"""

import numpy as np

import concourse.bass as bass
import concourse.bacc as bacc
import concourse.mybir as mybir
from concourse.tile import TileContext
from concourse.bass_utils import run_bass_kernel_spmd

P = 128
COLS = 32000  # 160*160*160 / 128
DELTA = 5.0
N_CORES = 8
CHUNK = 500  # PE reduction column-chunk (PSUM bank limit 512)

F32 = mybir.dt.float32
U32 = mybir.dt.uint32
ALU = mybir.AluOpType
ACTF = mybir.ActivationFunctionType


def build(cols=COLS, tile_f=2000, lead=(1000, 1000), tail=(1000, 500, 500)):
    main_cols = cols - sum(tail) - sum(lead)
    assert main_cols % tile_f == 0 and main_cols > 0
    tiles = list(lead) + [tile_f] * (main_cols // tile_f) + list(tail)
    assert all(f % CHUNK == 0 or f < CHUNK for f in tiles)
    n_elem = float(P * cols)
    n_tiles = len(tiles)
    total_mm = sum(max(1, f // CHUNK) for f in tiles)
    w_max = min(CHUNK, max(tiles))
    # first matmul carries start=True and must reset the widest PSUM region
    assert min(CHUNK, tiles[0]) == w_max

    nc = bacc.Bacc(
        "TRN2",
        target_bir_lowering=False,
        debug=False,
        enable_asserts=False,
        num_devices=N_CORES,
    )
    a_ext = nc.dram_tensor("y_pred_logits", [P, cols], F32, kind="ExternalInput")
    b_ext = nc.dram_tensor("y_true", [P, cols], F32, kind="ExternalInput")
    out_ext = nc.dram_tensor("out", [1, 1], F32, kind="ExternalOutput")

    with TileContext(nc) as tc:
        with (
            tc.tile_pool(name="io", bufs=5) as io_pool,
            tc.tile_pool(name="work", bufs=3) as work_pool,
            tc.tile_pool(name="acc", bufs=1) as acc_pool,
            tc.tile_pool(name="psum", bufs=1, space="PSUM") as psum_pool,
        ):
            sums_sq = acc_pool.tile([P, n_tiles], F32)
            sums_d2 = acc_pool.tile([P, n_tiles], F32)
            ones = acc_pool.tile([P, 1], F32)
            nc.vector.memset(ones[:], 1.0)
            psum_ae = psum_pool.tile([1, w_max], F32)

            col = 0
            mm_i = 0
            pending_m = None
            for t, f in enumerate(tiles):
                a = io_pool.tile([P, f], F32, tag="a")
                b = io_pool.tile([P, f], F32, tag="b")

                e = work_pool.tile([P, f], F32, tag="e")
                ae = work_pool.tile([P, f], F32, tag="ae")
                m = work_pool.tile([P, f], F32, tag="m")
                # dead stores (ACT must write a full tile alongside accum_out);
                # one shared tag keeps SBUF pressure down
                s_sq = work_pool.tile([P, f], F32, tag="dead")
                s_d2 = work_pool.tile([P, f], F32, tag="dead")

                sl = slice(col, col + f)
                col += f
                nc.sync.dma_start(out=a[:], in_=a_ext[:, sl])
                nc.sync.dma_start(out=b[:], in_=b_ext[:, sl])
                nc.vector.tensor_tensor(e[:], a[:], b[:], ALU.subtract)
                nc.vector.tensor_scalar(
                    ae.bitcast(U32)[:], e.bitcast(U32)[:],
                    0x7FFFFFFF, None, ALU.bitwise_and,
                )
                # m = max(|e|,5) - 5 == relu(|e|-5); 2x-mode tensor_scalar
                nc.vector.tensor_scalar(
                    m[:], ae[:], DELTA, -DELTA, ALU.max, ALU.add
                )
                nc.scalar.activation(
                    s_sq[:], e[:], ACTF.Square,
                    accum_out=sums_sq[:, t : t + 1],
                )
                # lag the m-square by one tile so ACT never waits on the
                # DVE abs->max chain of the current tile; no lag on the small
                # tail tiles (it would stretch the kernel's final chain)
                if pending_m is not None:
                    pm, ps, pt = pending_m
                    nc.scalar.activation(
                        ps[:], pm[:], ACTF.Square,
                        accum_out=sums_d2[:, pt : pt + 1],
                    )
                    pending_m = None
                if t < n_tiles - len(tail) - 1:
                    pending_m = (m, s_d2, t)
                else:
                    nc.scalar.activation(
                        s_d2[:], m[:], ACTF.Square,
                        accum_out=sums_d2[:, t : t + 1],
                    )
                for c in range(max(1, f // CHUNK)):
                    w = min(CHUNK, f - c * CHUNK)
                    nc.tensor.matmul(
                        psum_ae[0:1, 0:w], ones[:, 0:1],
                        ae[:, c * CHUNK : c * CHUNK + w],
                        start=(mm_i == 0), stop=(mm_i == total_mm - 1),
                    )
                    mm_i += 1
            assert pending_m is None
            assert col == cols and mm_i == total_mm

            red = acc_pool.tile([P, 2], F32)
            nc.vector.reduce_sum(red[:, 0:1], sums_sq[:], axis=mybir.AxisListType.X)
            nc.vector.reduce_sum(red[:, 1:2], sums_d2[:], axis=mybir.AxisListType.X)

            # partition reduce: ps[0,:] = ones^T @ red -> [S2, SR]
            ps = psum_pool.tile([1, 2], F32)
            nc.tensor.matmul(ps[0:1, 0:2], ones[:, 0:1], red[:, 0:2],
                             start=True, stop=True)

            sc = acc_pool.tile([1, 12], F32)
            l2 = sc[:, 0:1]
            srn = sc[:, 1:2]
            l1 = sc[:, 2:3]
            s1r = sc[:, 3:4]
            hub = sc[:, 4:5]
            l1sq = sc[:, 5:6]
            c1 = sc[:, 6:7]
            c2 = sc[:, 7:8]
            cond = sc[:, 8:9]
            dif = sc[:, 9:10]
            mm = sc[:, 10:11]
            per = sc[:, 11:12]

            # [L2, SR/N] in one scaled copy; S1 via the PE accumulator
            nc.scalar.activation(sc[:, 0:2], ps[0:1, 0:2], ACTF.Copy,
                                 scale=1.0 / n_elem)
            nc.vector.reduce_sum(s1r, psum_ae[0:1, :], axis=mybir.AxisListType.X)
            nc.scalar.activation(l1, s1r, ACTF.Copy, scale=1.0 / n_elem)
            # hub = 0.5 * (L2 - SR/N)
            nc.vector.tensor_tensor(dif, l2, srn, ALU.subtract)
            nc.scalar.activation(hub, dif, ACTF.Copy, scale=0.5)
            nc.vector.tensor_tensor(l1sq, l1, l1, ALU.mult)
            nc.vector.tensor_scalar(c1, l2, 1.0, None, ALU.is_le)
            nc.vector.tensor_tensor(c2, l2, l1sq, ALU.is_lt)
            nc.vector.tensor_tensor(cond, c1, c2, ALU.max)
            # per = hub + cond * (l2 - hub)
            nc.vector.tensor_tensor(dif, l2, hub, ALU.subtract)
            nc.vector.tensor_tensor(mm, cond, dif, ALU.mult)
            nc.vector.tensor_tensor(per, hub, mm, ALU.add)
            # Each core emits its own per-sample loss; the host averages the
            # 8 scalars during unshard.
            nc.sync.dma_start(out=out_ext[:, :], in_=per)

    nc.compile()
    return nc


_NC_CACHE = {}


def _get_nc():
    if "nc" not in _NC_CACHE:
        _NC_CACHE["nc"] = build()
    return _NC_CACHE["nc"]


def kernel(y_pred_logits: np.ndarray, y_true: np.ndarray, _trace=False) -> np.ndarray:
    nc = _get_nc()
    a = np.ascontiguousarray(y_pred_logits, dtype=np.float32).reshape(N_CORES, P, COLS)
    b = np.ascontiguousarray(y_true, dtype=np.float32).reshape(N_CORES, P, COLS)
    in_maps = [
        {"y_pred_logits": a[i], "y_true": b[i]} for i in range(N_CORES)
    ]
    r = run_bass_kernel_spmd(nc, in_maps, core_ids=list(range(N_CORES)), trace=_trace)
    per_sample = np.array(
        [np.asarray(r.results[i]["out"]).reshape(()) for i in range(N_CORES)],
        dtype=np.float32,
    )
    out = np.float32(per_sample.mean(dtype=np.float32)).reshape(())
    if _trace:
        return out, r
    return out


# revision 30
# speedup vs baseline: 1.0301x; 1.0301x over previous
"""Adaptive Huber/MSE/L1 loss on 8 TRN2 NeuronCores (Bass/Tile).

Reference math (per sample, N = 4,096,000 elements):
    e   = pred - true
    L2  = mean(e^2);  L1 = mean(|e|)
    huber_elem = where(|e| <= 5, 0.5 e^2, 5(|e| - 2.5))
               = 0.5 e^2 - 0.5 relu(|e| - 5)^2
    huber = (S2 - SR) * 0.5 / N        (S2 = sum e^2, SR = sum relu(|e|-5)^2)
    use_l2 = (L2 <= 1) | (L2 < L1^2)
    loss = mean_over_batch(where(use_l2, L2, huber))

Sharding: data-parallel, sample i -> core i. Each core reduces its
32.8 MB shard to three sums and applies the branch locally; the host
averages the 8 per-sample scalars during unshard (an on-device 4-byte
AllReduce costs ~42 us of pure latency).

Per-tile engine split, sized so every engine sits ~20% under the
~358 GB/s-per-core DMA floor (5.86 us per 2 MB tile pair):
    DVE : e = a - b;  |e| = e & 0x7fffffff (uint32 bitcast, 2x mode);
          m = max(|e|,5) - 5 (fused tensor_scalar, 2x mode)
    ACT : Square(e) + row-accum (S2);  Square(m) + row-accum (SR)
    PE  : ones^T @ |e| chunks accumulated in PSUM (S1), plus the final
          [P,2] -> [1,2] partition reduction
The trailing columns use small tiles so the last dependency chain is
short and the kernel tail hugs the final DMA.

Hardware pitfalls baked in: DVE tensor_tensor_reduce and tensor_scalar
abs_max/accum_out fail on this toolchain (avoided); GpSimd elementwise
runs ~30 us/tile AND port-starves DVE (avoided); profiling must capture
all 8 devices (see test harness).
"""

import numpy as np

import concourse.bass as bass
import concourse.bacc as bacc
import concourse.mybir as mybir
from concourse.tile import TileContext
from concourse.bass_utils import run_bass_kernel_spmd

P = 128
COLS = 32000  # 160*160*160 / 128
DELTA = 5.0
N_CORES = 8
CHUNK = 500  # PE reduction column-chunk (PSUM bank limit 512)

F32 = mybir.dt.float32
U32 = mybir.dt.uint32
ALU = mybir.AluOpType
ACTF = mybir.ActivationFunctionType


def build(cols=COLS, tile_f=2000, lead=(1000, 1000), tail=(1000, 500, 500)):
    main_cols = cols - sum(tail) - sum(lead)
    assert main_cols % tile_f == 0 and main_cols > 0
    tiles = list(lead) + [tile_f] * (main_cols // tile_f) + list(tail)
    assert all(f % CHUNK == 0 or f < CHUNK for f in tiles)
    n_elem = float(P * cols)
    n_tiles = len(tiles)
    total_mm = sum(max(1, f // CHUNK) for f in tiles)
    w_max = min(CHUNK, max(tiles))
    # first matmul carries start=True and must reset the widest PSUM region
    assert min(CHUNK, tiles[0]) == w_max

    nc = bacc.Bacc(
        "TRN2",
        target_bir_lowering=False,
        debug=False,
        enable_asserts=False,
        num_devices=N_CORES,
    )
    a_ext = nc.dram_tensor("y_pred_logits", [P, cols], F32, kind="ExternalInput")
    b_ext = nc.dram_tensor("y_true", [P, cols], F32, kind="ExternalInput")
    out_ext = nc.dram_tensor("out", [1, 1], F32, kind="ExternalOutput")

    with TileContext(nc) as tc:
        with (
            tc.tile_pool(name="io", bufs=5) as io_pool,
            tc.tile_pool(name="work", bufs=3) as work_pool,
            tc.tile_pool(name="acc", bufs=1) as acc_pool,
            tc.tile_pool(name="psum", bufs=1, space="PSUM") as psum_pool,
        ):
            sums_sq = acc_pool.tile([P, n_tiles], F32)
            sums_d2 = acc_pool.tile([P, n_tiles], F32)
            ones = acc_pool.tile([P, 1], F32)
            nc.vector.memset(ones[:], 1.0)
            psum_ae = psum_pool.tile([1, w_max], F32)

            col = 0
            mm_i = 0
            pending_m = None
            for t, f in enumerate(tiles):
                a = io_pool.tile([P, f], F32, tag="a")
                b = io_pool.tile([P, f], F32, tag="b")

                e = work_pool.tile([P, f], F32, tag="e")
                ae = work_pool.tile([P, f], F32, tag="ae")
                m = work_pool.tile([P, f], F32, tag="m")
                s_sq = work_pool.tile([P, f], F32, tag="s_sq")
                s_d2 = work_pool.tile([P, f], F32, tag="s_d2")

                sl = slice(col, col + f)
                col += f
                nc.sync.dma_start(out=a[:], in_=a_ext[:, sl])
                nc.sync.dma_start(out=b[:], in_=b_ext[:, sl])
                nc.vector.tensor_tensor(e[:], a[:], b[:], ALU.subtract)
                nc.vector.tensor_scalar(
                    ae.bitcast(U32)[:], e.bitcast(U32)[:],
                    0x7FFFFFFF, None, ALU.bitwise_and,
                )
                # m = max(|e|,5) - 5 == relu(|e|-5); 2x-mode tensor_scalar
                nc.vector.tensor_scalar(
                    m[:], ae[:], DELTA, -DELTA, ALU.max, ALU.add
                )
                nc.scalar.activation(
                    s_sq[:], e[:], ACTF.Square,
                    accum_out=sums_sq[:, t : t + 1],
                )
                # lag the m-square by one tile so ACT never waits on the
                # DVE abs->max chain of the current tile; no lag on the small
                # tail tiles (it would stretch the kernel's final chain)
                if pending_m is not None:
                    pm, ps, pt = pending_m
                    nc.scalar.activation(
                        ps[:], pm[:], ACTF.Square,
                        accum_out=sums_d2[:, pt : pt + 1],
                    )
                    pending_m = None
                if t < n_tiles - len(tail) - 1:
                    pending_m = (m, s_d2, t)
                else:
                    nc.scalar.activation(
                        s_d2[:], m[:], ACTF.Square,
                        accum_out=sums_d2[:, t : t + 1],
                    )
                for c in range(max(1, f // CHUNK)):
                    w = min(CHUNK, f - c * CHUNK)
                    nc.tensor.matmul(
                        psum_ae[0:1, 0:w], ones[:, 0:1],
                        ae[:, c * CHUNK : c * CHUNK + w],
                        start=(mm_i == 0), stop=(mm_i == total_mm - 1),
                    )
                    mm_i += 1
            assert pending_m is None
            assert col == cols and mm_i == total_mm

            red = acc_pool.tile([P, 2], F32)
            nc.vector.reduce_sum(red[:, 0:1], sums_sq[:], axis=mybir.AxisListType.X)
            nc.vector.reduce_sum(red[:, 1:2], sums_d2[:], axis=mybir.AxisListType.X)

            # partition reduce: ps[0,:] = ones^T @ red -> [S2, SR]
            ps = psum_pool.tile([1, 2], F32)
            nc.tensor.matmul(ps[0:1, 0:2], ones[:, 0:1], red[:, 0:2],
                             start=True, stop=True)

            sc = acc_pool.tile([1, 12], F32)
            l2 = sc[:, 0:1]
            srn = sc[:, 1:2]
            l1 = sc[:, 2:3]
            s1r = sc[:, 3:4]
            hub = sc[:, 4:5]
            l1sq = sc[:, 5:6]
            c1 = sc[:, 6:7]
            c2 = sc[:, 7:8]
            cond = sc[:, 8:9]
            dif = sc[:, 9:10]
            mm = sc[:, 10:11]
            per = sc[:, 11:12]

            # [L2, SR/N] in one scaled copy; S1 via the PE accumulator
            nc.scalar.activation(sc[:, 0:2], ps[0:1, 0:2], ACTF.Copy,
                                 scale=1.0 / n_elem)
            nc.vector.reduce_sum(s1r, psum_ae[0:1, :], axis=mybir.AxisListType.X)
            nc.scalar.activation(l1, s1r, ACTF.Copy, scale=1.0 / n_elem)
            # hub = 0.5 * (L2 - SR/N)
            nc.vector.tensor_tensor(dif, l2, srn, ALU.subtract)
            nc.scalar.activation(hub, dif, ACTF.Copy, scale=0.5)
            nc.vector.tensor_tensor(l1sq, l1, l1, ALU.mult)
            nc.vector.tensor_scalar(c1, l2, 1.0, None, ALU.is_le)
            nc.vector.tensor_tensor(c2, l2, l1sq, ALU.is_lt)
            nc.vector.tensor_tensor(cond, c1, c2, ALU.max)
            # per = hub + cond * (l2 - hub)
            nc.vector.tensor_tensor(dif, l2, hub, ALU.subtract)
            nc.vector.tensor_tensor(mm, cond, dif, ALU.mult)
            nc.vector.tensor_tensor(per, hub, mm, ALU.add)
            # Each core emits its own per-sample loss; the host averages the
            # 8 scalars during unshard.
            nc.sync.dma_start(out=out_ext[:, :], in_=per)

    nc.compile()
    return nc


_NC_CACHE = {}


def _get_nc():
    if "nc" not in _NC_CACHE:
        _NC_CACHE["nc"] = build()
    return _NC_CACHE["nc"]


def kernel(y_pred_logits: np.ndarray, y_true: np.ndarray, _trace=False) -> np.ndarray:
    nc = _get_nc()
    a = np.ascontiguousarray(y_pred_logits, dtype=np.float32).reshape(N_CORES, P, COLS)
    b = np.ascontiguousarray(y_true, dtype=np.float32).reshape(N_CORES, P, COLS)
    in_maps = [
        {"y_pred_logits": a[i], "y_true": b[i]} for i in range(N_CORES)
    ]
    r = run_bass_kernel_spmd(nc, in_maps, core_ids=list(range(N_CORES)), trace=_trace)
    per_sample = np.array(
        [np.asarray(r.results[i]["out"]).reshape(()) for i in range(N_CORES)],
        dtype=np.float32,
    )
    out = np.float32(per_sample.mean(dtype=np.float32)).reshape(())
    if _trace:
        return out, r
    return out
